# revision 1
# baseline (speedup 1.0000x reference)
"""Trainium2 Bass kernel for a 4-layer MLP over N=100000 rows (DHGCN forward).

Reference computation (the graph edge_index `g` is dead):
    h = relu(x @ W0 + b0); h = relu(h @ W1 + b1)
    h = relu(h @ W2 + b2); h = relu(h @ W3 + b3)
with x [100000, 3000], W0 [3000,512], W1/W2 [512,512], W3 [512,20].

Strategy: data-parallel over rows across 8 NeuronCores (weights replicated).
On host, x is transposed to feature-major (xT) and the feature dim padded
3000 -> 3072 = 24*128 so activations live on-chip as [feat_part, row] tiles;
every matmul is then out[M=out_feat_chunk, N=rows] = W_tile.T @ hT_tile with
natural-layout weights and no on-device transposes.

Matmul operands use dtype float32r (same bits as fp32): plain-fp32 matmul
runs at 1/4 PE rate on TRN2, while f32r streams 1 row/cycle (full rate) at
free dim >= 256, at ~tf32 effective precision (measured rel err ~3e-4 for
the full 4-layer chain). The BIR verifier requires every producer feeding an
f32r matmul to emit f32r itself, so the x/weight DRAM tensors, their SBUF
tiles, and the relu-activation outputs (h tiles) are all declared f32r;
PSUM accumulation stays fp32 and the final output is fp32.
"""

import numpy as np

import concourse.bacc as bacc
import concourse.mybir as mybir
import concourse.tile as tile
from concourse.bass import ts
from concourse.bass_utils import run_bass_kernel_spmd

F32 = mybir.dt.float32
F32R = mybir.dt.float32r
RELU = mybir.ActivationFunctionType.Relu

N_CORES = 8
N_ROWS = 100000
ROWS_PER_CORE = N_ROWS // N_CORES  # 12500
R = 500                            # row-block (PSUM free dim <= 512)
N_BLK = ROWS_PER_CORE // R         # 25
IN_DIM = 3000
K0 = 3072                          # padded in_dim = 24*128
KT0 = K0 // 128                    # 24 K-tiles for layer 0
H = 512
KT = H // 128                      # 4 K-tiles for layers 1-3
M_CH = H // 128                    # 4 output chunks of 128 for layers 0-2
LAT = 20


def build_program():
    nc = bacc.Bacc("TRN2", target_bir_lowering=False, debug=False)

    xT = nc.dram_tensor("xT", [K0, ROWS_PER_CORE], F32R, kind="ExternalInput")
    w0 = nc.dram_tensor("w0", [K0, H], F32R, kind="ExternalInput")
    w1 = nc.dram_tensor("w1", [H, H], F32R, kind="ExternalInput")
    w2 = nc.dram_tensor("w2", [H, H], F32R, kind="ExternalInput")
    w3 = nc.dram_tensor("w3", [H, LAT], F32R, kind="ExternalInput")
    b0 = nc.dram_tensor("b0", [H], F32, kind="ExternalInput")
    b1 = nc.dram_tensor("b1", [H], F32, kind="ExternalInput")
    b2 = nc.dram_tensor("b2", [H], F32, kind="ExternalInput")
    b3 = nc.dram_tensor("b3", [LAT], F32, kind="ExternalInput")
    outT = nc.dram_tensor("outT", [LAT, ROWS_PER_CORE], F32, kind="ExternalOutput")

    xr = xT.rearrange("(ko p) r -> p ko r", p=128)    # [128, 24, 12500]
    w0r = w0.rearrange("(ko p) f -> p ko f", p=128)   # [128, 24, 512]
    w1r = w1.rearrange("(ko p) f -> p ko f", p=128)   # [128, 4, 512]
    w2r = w2.rearrange("(ko p) f -> p ko f", p=128)
    w3r = w3.rearrange("(ko p) f -> p ko f", p=128)   # [128, 4, 20]
    b0r = b0.rearrange("(m p) -> p m", p=128)         # [128, 4]
    b1r = b1.rearrange("(m p) -> p m", p=128)
    b2r = b2.rearrange("(m p) -> p m", p=128)
    b3r = b3.rearrange("(m p) -> p m", p=LAT)         # [20, 1]

    with tile.TileContext(nc) as tc:
        with (
            tc.tile_pool(name="const", bufs=1) as const,
            tc.tile_pool(name="xin", bufs=2) as xin,
            tc.tile_pool(name="hbuf", bufs=1) as hbuf,
            tc.tile_pool(name="obuf", bufs=2) as obuf,
            tc.tile_pool(name="psA", bufs=4, space="PSUM") as psA,
            tc.tile_pool(name="psB", bufs=2, space="PSUM") as psB,
        ):
            w0_sb = const.tile([128, KT0, H], F32R, tag="w0")
            w1_sb = const.tile([128, KT, H], F32R, tag="w1")
            w2_sb = const.tile([128, KT, H], F32R, tag="w2")
            w3_sb = const.tile([128, KT, LAT], F32R, tag="w3")
            b0_sb = const.tile([128, M_CH], F32, tag="b0")
            b1_sb = const.tile([128, M_CH], F32, tag="b1")
            b2_sb = const.tile([128, M_CH], F32, tag="b2")
            b3_sb = const.tile([LAT, 1], F32, tag="b3")
            for ko_w in range(KT0):
                nc.sync.dma_start(w0_sb[:, ko_w, :], w0r[:, ko_w, :])
            nc.sync.dma_start(w1_sb[:], w1r[:])
            nc.sync.dma_start(w2_sb[:], w2r[:])
            nc.sync.dma_start(w3_sb[:], w3r[:])
            nc.sync.dma_start(b0_sb[:], b0r[:])
            nc.sync.dma_start(b1_sb[:], b1r[:])
            nc.sync.dma_start(b2_sb[:], b2r[:])
            nc.sync.dma_start(b3_sb[:], b3r[:])

            for j in range(N_BLK):
                x_t = xin.tile([128, KT0, R], F32R, tag="x")
                if j == 0:
                    # Split w0 and block-0 x into per-K-tile DMAs so the first
                    # matmul group starts once its first tiles land instead of
                    # waiting out the full 12 MB startup load (-31 us model).
                    for ko in range(KT0):
                        nc.sync.dma_start(x_t[:, ko, :], xr[:, ko, ts(j, R)])
                else:
                    nc.sync.dma_start(x_t[:], xr[:, :, ts(j, R)])

                h1 = hbuf.tile([128, KT, R], F32R, tag="h1")
                for m in range(M_CH):
                    ps = psA.tile([128, R], F32, tag="ps")
                    for ko in range(KT0):
                        nc.tensor.matmul(
                            ps[:],
                            w0_sb[:, ko, ts(m, 128)],
                            x_t[:, ko, :],
                            start=(ko == 0),
                            stop=(ko == KT0 - 1),
                        )
                    nc.scalar.activation(
                        h1[:, m, :], ps[:], RELU, bias=b0_sb[:, m : m + 1]
                    )

                h2 = hbuf.tile([128, KT, R], F32R, tag="h2")
                for m in range(M_CH):
                    ps = psA.tile([128, R], F32, tag="ps")
                    for ko in range(KT):
                        nc.tensor.matmul(
                            ps[:],
                            w1_sb[:, ko, ts(m, 128)],
                            h1[:, ko, :],
                            start=(ko == 0),
                            stop=(ko == KT - 1),
                        )
                    nc.scalar.activation(
                        h2[:, m, :], ps[:], RELU, bias=b1_sb[:, m : m + 1]
                    )

                h3 = hbuf.tile([128, KT, R], F32R, tag="h3")
                for m in range(M_CH):
                    ps = psA.tile([128, R], F32, tag="ps")
                    for ko in range(KT):
                        nc.tensor.matmul(
                            ps[:],
                            w2_sb[:, ko, ts(m, 128)],
                            h2[:, ko, :],
                            start=(ko == 0),
                            stop=(ko == KT - 1),
                        )
                    nc.scalar.activation(
                        h3[:, m, :], ps[:], RELU, bias=b2_sb[:, m : m + 1]
                    )

                ps3 = psB.tile([LAT, R], F32, tag="ps3")
                for ko in range(KT):
                    nc.tensor.matmul(
                        ps3[:],
                        w3_sb[:, ko, :],
                        h3[:, ko, :],
                        start=(ko == 0),
                        stop=(ko == KT - 1),
                    )
                o_t = obuf.tile([LAT, R], F32, tag="o")
                nc.scalar.activation(o_t[:], ps3[:], RELU, bias=b3_sb[:])
                nc.sync.dma_start(outT[:, ts(j, R)], o_t[:])

    nc.compile()
    return nc


_NC = None


def _get_nc():
    global _NC
    if _NC is None:
        _NC = build_program()
    return _NC


def make_in_maps(inputs, W0, b0, W1, b1, W2, b2, W3, b3):
    """Host-side sharding: pad features to K0, transpose x to feature-major,
    slice rows across cores; weights replicated.

    Builds one [N_CORES*K0, ROWS_PER_CORE] buffer so each core's xT is a
    contiguous view (bass2jax's per-core np.asarray is then copy-free), and
    uses a row-blocked transpose (cache-friendlier than one big x.T assign).
    """
    x = np.asarray(inputs, dtype=np.float32)
    xT_cat = np.empty((N_CORES * K0, ROWS_PER_CORE), dtype=np.float32)
    RB = 3125  # transpose block: RB rows at a time
    for c in range(N_CORES):
        base = c * K0
        r0 = c * ROWS_PER_CORE
        for rb in range(0, ROWS_PER_CORE, RB):
            xT_cat[base : base + IN_DIM, rb : rb + RB] = x[
                r0 + rb : r0 + rb + RB
            ].T
        xT_cat[base + IN_DIM : base + K0] = 0.0
    w0p = np.zeros((K0, H), dtype=np.float32)
    w0p[:IN_DIM] = np.asarray(W0, dtype=np.float32)
    common = {
        "w0": w0p,
        "w1": np.ascontiguousarray(W1, dtype=np.float32),
        "w2": np.ascontiguousarray(W2, dtype=np.float32),
        "w3": np.ascontiguousarray(W3, dtype=np.float32),
        "b0": np.ascontiguousarray(b0, dtype=np.float32),
        "b1": np.ascontiguousarray(b1, dtype=np.float32),
        "b2": np.ascontiguousarray(b2, dtype=np.float32),
        "b3": np.ascontiguousarray(b3, dtype=np.float32),
    }
    in_maps = []
    for c in range(N_CORES):
        in_maps.append({"xT": xT_cat[c * K0 : (c + 1) * K0], **common})
    return in_maps


def kernel(inputs, g, W0, b0, W1, b1, W2, b2, W3, b3):
    nc = _get_nc()
    in_maps = make_in_maps(inputs, W0, b0, W1, b1, W2, b2, W3, b3)
    res = run_bass_kernel_spmd(nc, in_maps, core_ids=list(range(N_CORES)))
    out = np.empty((N_ROWS, LAT), dtype=np.float32)
    for c, r in enumerate(res.results):
        out[c * ROWS_PER_CORE : (c + 1) * ROWS_PER_CORE] = r["outT"].T
    return out



# revision 3
# speedup vs baseline: 2.8738x; 2.8738x over previous
"""Trainium2 Bass kernel for a 4-layer MLP over N=100000 rows (DHGCN forward).

Reference computation (the graph edge_index `g` is dead):
    h = relu(x @ W0 + b0); h = relu(h @ W1 + b1)
    h = relu(h @ W2 + b2); h = relu(h @ W3 + b3)
with x [100000, 3000], W0 [3000,512], W1/W2 [512,512], W3 [512,20].

Strategy: data-parallel over rows across 8 NeuronCores (weights replicated).
On host, x is transposed to feature-major (xT) and the feature dim padded
3000 -> 3072 = 24*128 so activations live on-chip as [feat_part, row] tiles;
every matmul is then out[M=out_feat_chunk, N=rows] = W_tile.T @ hT_tile with
natural-layout weights and no on-device transposes.

Matmul operands use dtype float32r (same bits as fp32): plain-fp32 matmul
runs at 1/4 PE rate on TRN2, while f32r streams 1 row/cycle (full rate) at
free dim >= 256, at ~tf32 effective precision (measured rel err ~3e-4 for
the full 4-layer chain). The BIR verifier requires every producer feeding an
f32r matmul to emit f32r itself, so the x/weight DRAM tensors, their SBUF
tiles, and the relu-activation outputs (h tiles) are all declared f32r;
PSUM accumulation stays fp32 and the final output is fp32.
"""

import numpy as np

import concourse.bacc as bacc
import concourse.mybir as mybir
import concourse.tile as tile
from concourse.bass import ts
from concourse.bass_utils import run_bass_kernel_spmd

F32 = mybir.dt.float32
F32R = mybir.dt.float32r
RELU = mybir.ActivationFunctionType.Relu

N_CORES = 8
N_ROWS = 100000
ROWS_PER_CORE = N_ROWS // N_CORES  # 12500
R = 500                            # row-block (PSUM free dim <= 512)
N_BLK = ROWS_PER_CORE // R         # 25
IN_DIM = 3000
K0 = 3072                          # padded in_dim = 24*128
KT0 = K0 // 128                    # 24 K-tiles for layer 0
H = 512
KT = H // 128                      # 4 K-tiles for layers 1-3
M_CH = H // 128                    # 4 output chunks of 128 for layers 0-2
LAT = 20


def build_program(iters: int = 1):
    """Build the per-core program. iters>1 repeats the full kernel body
    (weight loads included) back-to-back inside one NEFF; test.py uses the
    per-iteration slope between two iters values to measure steady-state HW
    exec time with host dispatch overhead differenced out. The graded
    kernel() path always uses iters=1."""
    nc = bacc.Bacc("TRN2", target_bir_lowering=False, debug=False)

    xT = nc.dram_tensor("xT", [K0, ROWS_PER_CORE], F32R, kind="ExternalInput")
    w0 = nc.dram_tensor("w0", [K0, H], F32R, kind="ExternalInput")
    w1 = nc.dram_tensor("w1", [H, H], F32R, kind="ExternalInput")
    w2 = nc.dram_tensor("w2", [H, H], F32R, kind="ExternalInput")
    w3 = nc.dram_tensor("w3", [H, LAT], F32R, kind="ExternalInput")
    b0 = nc.dram_tensor("b0", [H], F32, kind="ExternalInput")
    b1 = nc.dram_tensor("b1", [H], F32, kind="ExternalInput")
    b2 = nc.dram_tensor("b2", [H], F32, kind="ExternalInput")
    b3 = nc.dram_tensor("b3", [LAT], F32, kind="ExternalInput")
    outT = nc.dram_tensor("outT", [LAT, ROWS_PER_CORE], F32, kind="ExternalOutput")

    xr = xT.rearrange("(ko p) r -> p ko r", p=128)    # [128, 24, 12500]
    w0r = w0.rearrange("(ko p) f -> p ko f", p=128)   # [128, 24, 512]
    w1r = w1.rearrange("(ko p) f -> p ko f", p=128)   # [128, 4, 512]
    w2r = w2.rearrange("(ko p) f -> p ko f", p=128)
    w3r = w3.rearrange("(ko p) f -> p ko f", p=128)   # [128, 4, 20]
    b0r = b0.rearrange("(m p) -> p m", p=128)         # [128, 4]
    b1r = b1.rearrange("(m p) -> p m", p=128)
    b2r = b2.rearrange("(m p) -> p m", p=128)
    b3r = b3.rearrange("(m p) -> p m", p=LAT)         # [20, 1]

    with tile.TileContext(nc) as tc:
        with (
            tc.tile_pool(name="const", bufs=1) as const,
            tc.tile_pool(name="xin", bufs=2) as xin,
            tc.tile_pool(name="hbuf", bufs=1) as hbuf,
            tc.tile_pool(name="obuf", bufs=2) as obuf,
            tc.tile_pool(name="psA", bufs=4, space="PSUM") as psA,
            tc.tile_pool(name="psB", bufs=2, space="PSUM") as psB,
        ):
          for _ in range(iters):
            w0_sb = const.tile([128, KT0, H], F32R, tag="w0")
            w1_sb = const.tile([128, KT, H], F32R, tag="w1")
            w2_sb = const.tile([128, KT, H], F32R, tag="w2")
            w3_sb = const.tile([128, KT, LAT], F32R, tag="w3")
            b0_sb = const.tile([128, M_CH], F32, tag="b0")
            b1_sb = const.tile([128, M_CH], F32, tag="b1")
            b2_sb = const.tile([128, M_CH], F32, tag="b2")
            b3_sb = const.tile([LAT, 1], F32, tag="b3")
            for ko_w in range(KT0):
                nc.sync.dma_start(w0_sb[:, ko_w, :], w0r[:, ko_w, :])
            nc.sync.dma_start(w1_sb[:], w1r[:])
            nc.sync.dma_start(w2_sb[:], w2r[:])
            nc.sync.dma_start(w3_sb[:], w3r[:])
            nc.sync.dma_start(b0_sb[:], b0r[:])
            nc.sync.dma_start(b1_sb[:], b1r[:])
            nc.sync.dma_start(b2_sb[:], b2r[:])
            nc.sync.dma_start(b3_sb[:], b3r[:])

            for j in range(N_BLK):
                x_t = xin.tile([128, KT0, R], F32R, tag="x")
                if j == 0:
                    # Split w0 and block-0 x into per-K-tile DMAs so the first
                    # matmul group starts once its first tiles land instead of
                    # waiting out the full 12 MB startup load (-31 us model).
                    for ko in range(KT0):
                        nc.sync.dma_start(x_t[:, ko, :], xr[:, ko, ts(j, R)])
                else:
                    nc.sync.dma_start(x_t[:], xr[:, :, ts(j, R)])

                h1 = hbuf.tile([128, KT, R], F32R, tag="h1")
                for m in range(M_CH):
                    ps = psA.tile([128, R], F32, tag="ps")
                    for ko in range(KT0):
                        nc.tensor.matmul(
                            ps[:],
                            w0_sb[:, ko, ts(m, 128)],
                            x_t[:, ko, :],
                            start=(ko == 0),
                            stop=(ko == KT0 - 1),
                        )
                    nc.scalar.activation(
                        h1[:, m, :], ps[:], RELU, bias=b0_sb[:, m : m + 1]
                    )

                h2 = hbuf.tile([128, KT, R], F32R, tag="h2")
                for m in range(M_CH):
                    ps = psA.tile([128, R], F32, tag="ps")
                    for ko in range(KT):
                        nc.tensor.matmul(
                            ps[:],
                            w1_sb[:, ko, ts(m, 128)],
                            h1[:, ko, :],
                            start=(ko == 0),
                            stop=(ko == KT - 1),
                        )
                    nc.scalar.activation(
                        h2[:, m, :], ps[:], RELU, bias=b1_sb[:, m : m + 1]
                    )

                h3 = hbuf.tile([128, KT, R], F32R, tag="h3")
                for m in range(M_CH):
                    ps = psA.tile([128, R], F32, tag="ps")
                    for ko in range(KT):
                        nc.tensor.matmul(
                            ps[:],
                            w2_sb[:, ko, ts(m, 128)],
                            h2[:, ko, :],
                            start=(ko == 0),
                            stop=(ko == KT - 1),
                        )
                    nc.scalar.activation(
                        h3[:, m, :], ps[:], RELU, bias=b2_sb[:, m : m + 1]
                    )

                ps3 = psB.tile([LAT, R], F32, tag="ps3")
                for ko in range(KT):
                    nc.tensor.matmul(
                        ps3[:],
                        w3_sb[:, ko, :],
                        h3[:, ko, :],
                        start=(ko == 0),
                        stop=(ko == KT - 1),
                    )
                o_t = obuf.tile([LAT, R], F32, tag="o")
                nc.scalar.activation(o_t[:], ps3[:], RELU, bias=b3_sb[:])
                nc.sync.dma_start(outT[:, ts(j, R)], o_t[:])

    nc.compile()
    return nc


_NC = None


def _get_nc():
    global _NC
    if _NC is None:
        _NC = build_program()
    return _NC


def make_in_maps(inputs, W0, b0, W1, b1, W2, b2, W3, b3):
    """Host-side sharding: pad features to K0, transpose x to feature-major,
    slice rows across cores; weights replicated.

    Builds one [N_CORES*K0, ROWS_PER_CORE] buffer so each core's xT is a
    contiguous view (bass2jax's per-core np.asarray is then copy-free), and
    uses a row-blocked transpose (cache-friendlier than one big x.T assign).
    """
    x = np.asarray(inputs, dtype=np.float32)
    xT_cat = np.empty((N_CORES * K0, ROWS_PER_CORE), dtype=np.float32)
    RB = 3125  # transpose block: RB rows at a time
    for c in range(N_CORES):
        base = c * K0
        r0 = c * ROWS_PER_CORE
        for rb in range(0, ROWS_PER_CORE, RB):
            xT_cat[base : base + IN_DIM, rb : rb + RB] = x[
                r0 + rb : r0 + rb + RB
            ].T
        xT_cat[base + IN_DIM : base + K0] = 0.0
    w0p = np.zeros((K0, H), dtype=np.float32)
    w0p[:IN_DIM] = np.asarray(W0, dtype=np.float32)
    common = {
        "w0": w0p,
        "w1": np.ascontiguousarray(W1, dtype=np.float32),
        "w2": np.ascontiguousarray(W2, dtype=np.float32),
        "w3": np.ascontiguousarray(W3, dtype=np.float32),
        "b0": np.ascontiguousarray(b0, dtype=np.float32),
        "b1": np.ascontiguousarray(b1, dtype=np.float32),
        "b2": np.ascontiguousarray(b2, dtype=np.float32),
        "b3": np.ascontiguousarray(b3, dtype=np.float32),
    }
    in_maps = []
    for c in range(N_CORES):
        in_maps.append({"xT": xT_cat[c * K0 : (c + 1) * K0], **common})
    return in_maps


def kernel(inputs, g, W0, b0, W1, b1, W2, b2, W3, b3):
    nc = _get_nc()
    in_maps = make_in_maps(inputs, W0, b0, W1, b1, W2, b2, W3, b3)
    res = run_bass_kernel_spmd(nc, in_maps, core_ids=list(range(N_CORES)))
    out = np.empty((N_ROWS, LAT), dtype=np.float32)
    for c, r in enumerate(res.results):
        out[c * ROWS_PER_CORE : (c + 1) * ROWS_PER_CORE] = r["outT"].T
    return out



# revision 4
# speedup vs baseline: 3.1631x; 1.1006x over previous
"""Trainium2 Bass kernel for a 4-layer MLP over N=100000 rows (DHGCN forward).

Reference computation (the graph edge_index `g` is dead):
    h = relu(x @ W0 + b0); h = relu(h @ W1 + b1)
    h = relu(h @ W2 + b2); h = relu(h @ W3 + b3)
with x [100000, 3000], W0 [3000,512], W1/W2 [512,512], W3 [512,20].

Strategy: data-parallel over rows across 8 NeuronCores (weights replicated).
On host, x is transposed to feature-major (xT) and the feature dim padded
3000 -> 3072 = 24*128 so activations live on-chip as [feat_part, row] tiles;
every matmul is then out[M=out_feat_chunk, N=rows] = W_tile.T @ hT_tile with
natural-layout weights and no on-device transposes.

Matmul operands are bfloat16. HW-measured (iters-slope method, all
operands SBUF-resident): the f32r kernel streams at ~1.65 GHz effective
(950 us full kernel), bf16 at ~1.95 GHz (878 us) -- the PE's documented P0
power-state downclock caps sustained 8-core matmul at ~2.0 GHz, and f32r
pays an extra 4-byte weight-path penalty on top. N-sweep probes (N=512 vs
256 at equal streamed columns: 606 vs 603 us) show no per-matmul overhead,
so the remaining gap to the 2.4 GHz roofline is clock, not scheduling.
bf16 numerics: rel err 5.1e-3 vs the 2e-2 gate (PSUM accumulation stays
fp32; biases+relu applied at fp32; final output fp32).
"""

import numpy as np

import concourse.bacc as bacc
import concourse.mybir as mybir
import concourse.tile as tile
from concourse.bass import ts
from concourse.bass_utils import run_bass_kernel_spmd

F32 = mybir.dt.float32
F32R = mybir.dt.bfloat16  # matmul operand dtype (bf16: 1 row/cycle + FWL)
RELU = mybir.ActivationFunctionType.Relu

N_CORES = 8
N_ROWS = 100000
ROWS_PER_CORE = N_ROWS // N_CORES  # 12500
R = 500                            # row-block (PSUM free dim <= 512)
N_BLK = ROWS_PER_CORE // R         # 25
IN_DIM = 3000
K0 = 3072                          # padded in_dim = 24*128
KT0 = K0 // 128                    # 24 K-tiles for layer 0
H = 512
KT = H // 128                      # 4 K-tiles for layers 1-3
M_CH = H // 128                    # 4 output chunks of 128 for layers 0-2
LAT = 20


def build_program(iters: int = 1):
    """Build the per-core program. iters>1 repeats the full kernel body
    (weight loads included) back-to-back inside one NEFF; test.py uses the
    per-iteration slope between two iters values to measure steady-state HW
    exec time with host dispatch overhead differenced out. The graded
    kernel() path always uses iters=1."""
    nc = bacc.Bacc("TRN2", target_bir_lowering=False, debug=False)

    xT = nc.dram_tensor("xT", [K0, ROWS_PER_CORE], F32R, kind="ExternalInput")
    w0 = nc.dram_tensor("w0", [K0, H], F32R, kind="ExternalInput")
    w1 = nc.dram_tensor("w1", [H, H], F32R, kind="ExternalInput")
    w2 = nc.dram_tensor("w2", [H, H], F32R, kind="ExternalInput")
    w3 = nc.dram_tensor("w3", [H, LAT], F32R, kind="ExternalInput")
    b0 = nc.dram_tensor("b0", [H], F32, kind="ExternalInput")
    b1 = nc.dram_tensor("b1", [H], F32, kind="ExternalInput")
    b2 = nc.dram_tensor("b2", [H], F32, kind="ExternalInput")
    b3 = nc.dram_tensor("b3", [LAT], F32, kind="ExternalInput")
    outT = nc.dram_tensor("outT", [LAT, ROWS_PER_CORE], F32, kind="ExternalOutput")

    xr = xT.rearrange("(ko p) r -> p ko r", p=128)    # [128, 24, 12500]
    w0r = w0.rearrange("(ko p) f -> p ko f", p=128)   # [128, 24, 512]
    w1r = w1.rearrange("(ko p) f -> p ko f", p=128)   # [128, 4, 512]
    w2r = w2.rearrange("(ko p) f -> p ko f", p=128)
    w3r = w3.rearrange("(ko p) f -> p ko f", p=128)   # [128, 4, 20]
    b0r = b0.rearrange("(m p) -> p m", p=128)         # [128, 4]
    b1r = b1.rearrange("(m p) -> p m", p=128)
    b2r = b2.rearrange("(m p) -> p m", p=128)
    b3r = b3.rearrange("(m p) -> p m", p=LAT)         # [20, 1]

    with tile.TileContext(nc) as tc:
        with (
            tc.tile_pool(name="const", bufs=1) as const,
            tc.tile_pool(name="xin", bufs=2) as xin,
            tc.tile_pool(name="hbuf", bufs=1) as hbuf,
            tc.tile_pool(name="obuf", bufs=2) as obuf,
            tc.tile_pool(name="psA", bufs=4, space="PSUM") as psA,
            tc.tile_pool(name="psB", bufs=2, space="PSUM") as psB,
        ):
          for _ in range(iters):
            w0_sb = const.tile([128, KT0, H], F32R, tag="w0")
            w1_sb = const.tile([128, KT, H], F32R, tag="w1")
            w2_sb = const.tile([128, KT, H], F32R, tag="w2")
            w3_sb = const.tile([128, KT, LAT], F32R, tag="w3")
            b0_sb = const.tile([128, M_CH], F32, tag="b0")
            b1_sb = const.tile([128, M_CH], F32, tag="b1")
            b2_sb = const.tile([128, M_CH], F32, tag="b2")
            b3_sb = const.tile([LAT, 1], F32, tag="b3")
            for ko_w in range(KT0):
                nc.sync.dma_start(w0_sb[:, ko_w, :], w0r[:, ko_w, :])
            nc.sync.dma_start(w1_sb[:], w1r[:])
            nc.sync.dma_start(w2_sb[:], w2r[:])
            nc.sync.dma_start(w3_sb[:], w3r[:])
            nc.sync.dma_start(b0_sb[:], b0r[:])
            nc.sync.dma_start(b1_sb[:], b1r[:])
            nc.sync.dma_start(b2_sb[:], b2r[:])
            nc.sync.dma_start(b3_sb[:], b3r[:])

            for j in range(N_BLK):
                x_t = xin.tile([128, KT0, R], F32R, tag="x")
                if j == 0:
                    # Split w0 and block-0 x into per-K-tile DMAs so the first
                    # matmul group starts once its first tiles land instead of
                    # waiting out the full 12 MB startup load (-31 us model).
                    for ko in range(KT0):
                        nc.sync.dma_start(x_t[:, ko, :], xr[:, ko, ts(j, R)])
                else:
                    nc.sync.dma_start(x_t[:], xr[:, :, ts(j, R)])

                h1 = hbuf.tile([128, KT, R], F32R, tag="h1")
                for m in range(M_CH):
                    ps = psA.tile([128, R], F32, tag="ps")
                    for ko in range(KT0):
                        nc.tensor.matmul(
                            ps[:],
                            w0_sb[:, ko, ts(m, 128)],
                            x_t[:, ko, :],
                            start=(ko == 0),
                            stop=(ko == KT0 - 1),
                        )
                    nc.scalar.activation(
                        h1[:, m, :], ps[:], RELU, bias=b0_sb[:, m : m + 1]
                    )

                h2 = hbuf.tile([128, KT, R], F32R, tag="h2")
                for m in range(M_CH):
                    ps = psA.tile([128, R], F32, tag="ps")
                    for ko in range(KT):
                        nc.tensor.matmul(
                            ps[:],
                            w1_sb[:, ko, ts(m, 128)],
                            h1[:, ko, :],
                            start=(ko == 0),
                            stop=(ko == KT - 1),
                        )
                    nc.scalar.activation(
                        h2[:, m, :], ps[:], RELU, bias=b1_sb[:, m : m + 1]
                    )

                h3 = hbuf.tile([128, KT, R], F32R, tag="h3")
                for m in range(M_CH):
                    ps = psA.tile([128, R], F32, tag="ps")
                    for ko in range(KT):
                        nc.tensor.matmul(
                            ps[:],
                            w2_sb[:, ko, ts(m, 128)],
                            h2[:, ko, :],
                            start=(ko == 0),
                            stop=(ko == KT - 1),
                        )
                    nc.scalar.activation(
                        h3[:, m, :], ps[:], RELU, bias=b2_sb[:, m : m + 1]
                    )

                ps3 = psB.tile([LAT, R], F32, tag="ps3")
                for ko in range(KT):
                    nc.tensor.matmul(
                        ps3[:],
                        w3_sb[:, ko, :],
                        h3[:, ko, :],
                        start=(ko == 0),
                        stop=(ko == KT - 1),
                    )
                o_t = obuf.tile([LAT, R], F32, tag="o")
                nc.scalar.activation(o_t[:], ps3[:], RELU, bias=b3_sb[:])
                nc.sync.dma_start(outT[:, ts(j, R)], o_t[:])

    nc.compile()
    return nc


_NC = None


def _get_nc():
    global _NC
    if _NC is None:
        _NC = build_program()
    return _NC


def make_in_maps(inputs, W0, b0, W1, b1, W2, b2, W3, b3):
    """Host-side sharding: pad features to K0, transpose x to feature-major,
    slice rows across cores; weights replicated.

    Builds one [N_CORES*K0, ROWS_PER_CORE] buffer so each core's xT is a
    contiguous view (bass2jax's per-core np.asarray is then copy-free), and
    uses a row-blocked transpose (cache-friendlier than one big x.T assign).
    """
    import ml_dtypes

    bf16 = np.dtype(ml_dtypes.bfloat16)
    x = np.asarray(inputs, dtype=np.float32).astype(bf16)
    xT_cat = np.empty((N_CORES * K0, ROWS_PER_CORE), dtype=bf16)
    RB = 3125  # transpose block: RB rows at a time
    for c in range(N_CORES):
        base = c * K0
        r0 = c * ROWS_PER_CORE
        for rb in range(0, ROWS_PER_CORE, RB):
            xT_cat[base : base + IN_DIM, rb : rb + RB] = x[
                r0 + rb : r0 + rb + RB
            ].T
        xT_cat[base + IN_DIM : base + K0] = 0.0
    w0p = np.zeros((K0, H), dtype=bf16)
    w0p[:IN_DIM] = np.asarray(W0, dtype=np.float32).astype(bf16)
    common = {
        "w0": w0p,
        "w1": np.ascontiguousarray(W1, dtype=np.float32).astype(bf16),
        "w2": np.ascontiguousarray(W2, dtype=np.float32).astype(bf16),
        "w3": np.ascontiguousarray(W3, dtype=np.float32).astype(bf16),
        "b0": np.ascontiguousarray(b0, dtype=np.float32),
        "b1": np.ascontiguousarray(b1, dtype=np.float32),
        "b2": np.ascontiguousarray(b2, dtype=np.float32),
        "b3": np.ascontiguousarray(b3, dtype=np.float32),
    }
    in_maps = []
    for c in range(N_CORES):
        in_maps.append({"xT": xT_cat[c * K0 : (c + 1) * K0], **common})
    return in_maps


def kernel(inputs, g, W0, b0, W1, b1, W2, b2, W3, b3):
    nc = _get_nc()
    in_maps = make_in_maps(inputs, W0, b0, W1, b1, W2, b2, W3, b3)
    res = run_bass_kernel_spmd(nc, in_maps, core_ids=list(range(N_CORES)))
    out = np.empty((N_ROWS, LAT), dtype=np.float32)
    for c, r in enumerate(res.results):
        out[c * ROWS_PER_CORE : (c + 1) * ROWS_PER_CORE] = r["outT"].T
    return out



# revision 5
# speedup vs baseline: 3.1914x; 1.0090x over previous
"""Trainium2 Bass kernel for a 4-layer MLP over N=100000 rows (DHGCN forward).

Reference computation (the graph edge_index `g` is dead):
    h = relu(x @ W0 + b0); h = relu(h @ W1 + b1)
    h = relu(h @ W2 + b2); h = relu(h @ W3 + b3)
with x [100000, 3000], W0 [3000,512], W1/W2 [512,512], W3 [512,20].

Strategy: data-parallel over rows across 8 NeuronCores (weights replicated).
On host, x is transposed to feature-major (xT) and the feature dim padded
3000 -> 3072 = 24*128 so activations live on-chip as [feat_part, row] tiles;
every matmul is then out[M=out_feat_chunk, N=rows] = W_tile.T @ hT_tile with
natural-layout weights and no on-device transposes.

Matmul operands are bfloat16. HW-measured (iters-slope method, all
operands SBUF-resident): the f32r kernel streams at ~1.65 GHz effective
(950 us full kernel), bf16 at ~1.95 GHz (878 us) -- the PE's documented P0
power-state downclock caps sustained 8-core matmul at ~2.0 GHz, and f32r
pays an extra 4-byte weight-path penalty on top. N-sweep probes (N=512 vs
256 at equal streamed columns: 606 vs 603 us) show no per-matmul overhead,
so the remaining gap to the 2.4 GHz roofline is clock, not scheduling.
bf16 numerics: rel err 5.1e-3 vs the 2e-2 gate (PSUM accumulation stays
fp32; biases+relu applied at fp32; final output fp32).
"""

import numpy as np

import concourse.bacc as bacc
import concourse.mybir as mybir
import concourse.tile as tile
from concourse.bass import ts
from concourse.bass_utils import run_bass_kernel_spmd

F32 = mybir.dt.float32
F32R = mybir.dt.bfloat16  # matmul operand dtype (bf16: 1 row/cycle + FWL)
RELU = mybir.ActivationFunctionType.Relu

N_CORES = 8
N_ROWS = 100000
ROWS_PER_CORE = N_ROWS // N_CORES  # 12500
R = 500                            # row-block (PSUM free dim <= 512)
N_BLK = ROWS_PER_CORE // R         # 25
IN_DIM = 3000
K0 = 3072                          # padded in_dim = 24*128
KT0 = K0 // 128                    # 24 K-tiles for layer 0
H = 512
KT = H // 128                      # 4 K-tiles for layers 1-3
M_CH = H // 128                    # 4 output chunks of 128 for layers 0-2
LAT = 20


def build_program(iters: int = 1):
    """Build the per-core program. iters>1 repeats the full kernel body
    (weight loads included) back-to-back inside one NEFF; test.py uses the
    per-iteration slope between two iters values to measure steady-state HW
    exec time with host dispatch overhead differenced out. The graded
    kernel() path always uses iters=1."""
    nc = bacc.Bacc("TRN2", target_bir_lowering=False, debug=False)

    xT = nc.dram_tensor("xT", [K0, ROWS_PER_CORE], F32R, kind="ExternalInput")
    w0 = nc.dram_tensor("w0", [K0, H], F32R, kind="ExternalInput")
    w1 = nc.dram_tensor("w1", [H, H], F32R, kind="ExternalInput")
    w2 = nc.dram_tensor("w2", [H, H], F32R, kind="ExternalInput")
    w3 = nc.dram_tensor("w3", [H, LAT], F32R, kind="ExternalInput")
    b0 = nc.dram_tensor("b0", [H], F32, kind="ExternalInput")
    b1 = nc.dram_tensor("b1", [H], F32, kind="ExternalInput")
    b2 = nc.dram_tensor("b2", [H], F32, kind="ExternalInput")
    b3 = nc.dram_tensor("b3", [LAT], F32, kind="ExternalInput")
    outT = nc.dram_tensor("outT", [LAT, ROWS_PER_CORE], F32, kind="ExternalOutput")

    xr = xT.rearrange("(ko p) r -> p ko r", p=128)    # [128, 24, 12500]
    w0r = w0.rearrange("(ko p) f -> p ko f", p=128)   # [128, 24, 512]
    w1r = w1.rearrange("(ko p) f -> p ko f", p=128)   # [128, 4, 512]
    w2r = w2.rearrange("(ko p) f -> p ko f", p=128)
    w3r = w3.rearrange("(ko p) f -> p ko f", p=128)   # [128, 4, 20]
    b0r = b0.rearrange("(m p) -> p m", p=128)         # [128, 4]
    b1r = b1.rearrange("(m p) -> p m", p=128)
    b2r = b2.rearrange("(m p) -> p m", p=128)
    b3r = b3.rearrange("(m p) -> p m", p=LAT)         # [20, 1]

    with tile.TileContext(nc) as tc:
        with (
            tc.tile_pool(name="const", bufs=1) as const,
            tc.tile_pool(name="xin", bufs=2) as xin,
            tc.tile_pool(name="hbuf", bufs=1) as hbuf,
            tc.tile_pool(name="obuf", bufs=2) as obuf,
            tc.tile_pool(name="psA", bufs=4, space="PSUM") as psA,
            tc.tile_pool(name="psB", bufs=2, space="PSUM") as psB,
        ):
          for _ in range(iters):
            w0_sb = const.tile([128, KT0, H], F32R, tag="w0")
            w1_sb = const.tile([128, KT, H], F32R, tag="w1")
            w2_sb = const.tile([128, KT, H], F32R, tag="w2")
            w3_sb = const.tile([128, KT, LAT], F32R, tag="w3")
            b0_sb = const.tile([128, M_CH], F32, tag="b0")
            b1_sb = const.tile([128, M_CH], F32, tag="b1")
            b2_sb = const.tile([128, M_CH], F32, tag="b2")
            b3_sb = const.tile([LAT, 1], F32, tag="b3")
            # Interleave w0 and block-0 x per K-tile so the first accumulation
            # chain starts once pair 0 lands (~0.7 us) instead of after the
            # whole weight load; w1-w3/biases aren't needed until block-0 L0
            # finishes, so they queue behind.
            x0_t = xin.tile([128, KT0, R], F32R, tag="x", name="x0_t")
            for ko_w in range(KT0):
                nc.sync.dma_start(w0_sb[:, ko_w, :], w0r[:, ko_w, :])
                nc.sync.dma_start(x0_t[:, ko_w, :], xr[:, ko_w, ts(0, R)])
            nc.sync.dma_start(b0_sb[:], b0r[:])
            nc.sync.dma_start(w1_sb[:], w1r[:])
            nc.sync.dma_start(w2_sb[:], w2r[:])
            nc.sync.dma_start(w3_sb[:], w3r[:])
            nc.sync.dma_start(b1_sb[:], b1r[:])
            nc.sync.dma_start(b2_sb[:], b2r[:])
            nc.sync.dma_start(b3_sb[:], b3r[:])

            for j in range(N_BLK):
                if j == 0:
                    x_t = x0_t
                else:
                    x_t = xin.tile([128, KT0, R], F32R, tag="x", name="x_t")
                    nc.sync.dma_start(x_t[:], xr[:, :, ts(j, R)])

                h1 = hbuf.tile([128, KT, R], F32R, tag="h1")
                for m in range(M_CH):
                    ps = psA.tile([128, R], F32, tag="ps")
                    for ko in range(KT0):
                        nc.tensor.matmul(
                            ps[:],
                            w0_sb[:, ko, ts(m, 128)],
                            x_t[:, ko, :],
                            start=(ko == 0),
                            stop=(ko == KT0 - 1),
                        )
                    nc.scalar.activation(
                        h1[:, m, :], ps[:], RELU, bias=b0_sb[:, m : m + 1]
                    )

                h2 = hbuf.tile([128, KT, R], F32R, tag="h2")
                for m in range(M_CH):
                    ps = psA.tile([128, R], F32, tag="ps")
                    for ko in range(KT):
                        nc.tensor.matmul(
                            ps[:],
                            w1_sb[:, ko, ts(m, 128)],
                            h1[:, ko, :],
                            start=(ko == 0),
                            stop=(ko == KT - 1),
                        )
                    nc.scalar.activation(
                        h2[:, m, :], ps[:], RELU, bias=b1_sb[:, m : m + 1]
                    )

                h3 = hbuf.tile([128, KT, R], F32R, tag="h3")
                for m in range(M_CH):
                    ps = psA.tile([128, R], F32, tag="ps")
                    for ko in range(KT):
                        nc.tensor.matmul(
                            ps[:],
                            w2_sb[:, ko, ts(m, 128)],
                            h2[:, ko, :],
                            start=(ko == 0),
                            stop=(ko == KT - 1),
                        )
                    nc.scalar.activation(
                        h3[:, m, :], ps[:], RELU, bias=b2_sb[:, m : m + 1]
                    )

                ps3 = psB.tile([LAT, R], F32, tag="ps3")
                for ko in range(KT):
                    nc.tensor.matmul(
                        ps3[:],
                        w3_sb[:, ko, :],
                        h3[:, ko, :],
                        start=(ko == 0),
                        stop=(ko == KT - 1),
                    )
                o_t = obuf.tile([LAT, R], F32, tag="o")
                nc.scalar.activation(o_t[:], ps3[:], RELU, bias=b3_sb[:])
                nc.sync.dma_start(outT[:, ts(j, R)], o_t[:])

    nc.compile()
    return nc


_NC = None


def _get_nc():
    global _NC
    if _NC is None:
        _NC = build_program()
    return _NC


def make_in_maps(inputs, W0, b0, W1, b1, W2, b2, W3, b3):
    """Host-side sharding: pad features to K0, transpose x to feature-major,
    slice rows across cores; weights replicated.

    Builds one [N_CORES*K0, ROWS_PER_CORE] buffer so each core's xT is a
    contiguous view (bass2jax's per-core np.asarray is then copy-free), and
    uses a row-blocked transpose (cache-friendlier than one big x.T assign).
    """
    import ml_dtypes

    bf16 = np.dtype(ml_dtypes.bfloat16)
    x = np.asarray(inputs, dtype=np.float32).astype(bf16)
    xT_cat = np.empty((N_CORES * K0, ROWS_PER_CORE), dtype=bf16)
    RB = 3125  # transpose block: RB rows at a time
    for c in range(N_CORES):
        base = c * K0
        r0 = c * ROWS_PER_CORE
        for rb in range(0, ROWS_PER_CORE, RB):
            xT_cat[base : base + IN_DIM, rb : rb + RB] = x[
                r0 + rb : r0 + rb + RB
            ].T
        xT_cat[base + IN_DIM : base + K0] = 0.0
    w0p = np.zeros((K0, H), dtype=bf16)
    w0p[:IN_DIM] = np.asarray(W0, dtype=np.float32).astype(bf16)
    common = {
        "w0": w0p,
        "w1": np.ascontiguousarray(W1, dtype=np.float32).astype(bf16),
        "w2": np.ascontiguousarray(W2, dtype=np.float32).astype(bf16),
        "w3": np.ascontiguousarray(W3, dtype=np.float32).astype(bf16),
        "b0": np.ascontiguousarray(b0, dtype=np.float32),
        "b1": np.ascontiguousarray(b1, dtype=np.float32),
        "b2": np.ascontiguousarray(b2, dtype=np.float32),
        "b3": np.ascontiguousarray(b3, dtype=np.float32),
    }
    in_maps = []
    for c in range(N_CORES):
        in_maps.append({"xT": xT_cat[c * K0 : (c + 1) * K0], **common})
    return in_maps


def kernel(inputs, g, W0, b0, W1, b1, W2, b2, W3, b3):
    nc = _get_nc()
    in_maps = make_in_maps(inputs, W0, b0, W1, b1, W2, b2, W3, b3)
    res = run_bass_kernel_spmd(nc, in_maps, core_ids=list(range(N_CORES)))
    out = np.empty((N_ROWS, LAT), dtype=np.float32)
    for c, r in enumerate(res.results):
        out[c * ROWS_PER_CORE : (c + 1) * ROWS_PER_CORE] = r["outT"].T
    return out

